# revision 1
# baseline (speedup 1.0000x reference)
import numpy as np
from concourse import bacc, tile, mybir
from concourse import bass_utils

B, N, D, K = 8, 2048, 256, 16
EPS = 1e-5
CH = 64            # tokens per chunk
NCH = N // CH      # 32 chunks
FC = 512           # pairs per inner sub-chunk
TPF = FC // K      # 32 tokens per sub-chunk

f32 = mybir.dt.float32
f32r = mybir.dt.float32r
u16 = mybir.dt.uint16
i16 = mybir.dt.int16
AF = mybir.ActivationFunctionType
AX = mybir.AxisListType
OP = mybir.AluOpType

_NC = None


def _build():
    nc = bacc.Bacc("TRN2", target_bir_lowering=False, debug=False)
    xT_d = nc.dram_tensor("xT", [D, N], f32, kind="ExternalInput")
    pp_d = nc.dram_tensor("pospack", [16, N], f32, kind="ExternalInput")
    wqkv_d = nc.dram_tensor("wqkv", [D, 768], f32, kind="ExternalInput")
    wabcd_d = nc.dram_tensor("wabcd", [D, 1024], f32, kind="ExternalInput")
    sw_d = nc.dram_tensor("spw1T", [16, D], f32, kind="ExternalInput")
    bs_d = nc.dram_tensor("biases", [128, 6], f32, kind="ExternalInput")
    outT_d = nc.dram_tensor("outT", [D, N], f32, kind="ExternalOutput")

    with tile.TileContext(nc) as tc:
        with tc.tile_pool(name="pers", bufs=1) as P, tc.tile_pool(
            name="rtp", bufs=1
        ) as R, tc.tile_pool(name="gp", bufs=1) as GP, tc.tile_pool(
            name="fp", bufs=1
        ) as FP, tc.tile_pool(name="cp", bufs=2) as CP, tc.tile_pool(
            name="ps", bufs=1, space="PSUM"
        ) as PS:
            xs = [P.tile([128, N], f32r, name=f"xs{h}") for h in range(2)]
            wq = [P.tile([128, 768], f32r, name=f"wq{h}") for h in range(2)]
            wA = [P.tile([128, 1024], f32r, name=f"wA{h}") for h in range(2)]
            wf = [P.tile([128, 256], f32, name=f"wf{h}") for h in range(2)]
            for h in range(2):
                nc.sync.dma_start(xs[h][:], xT_d[128 * h : 128 * (h + 1), :].bitcast(f32r))
                nc.sync.dma_start(wq[h][:], wqkv_d[128 * h : 128 * (h + 1), :].bitcast(f32r))
                nc.sync.dma_start(wA[h][:], wabcd_d[128 * h : 128 * (h + 1), :].bitcast(f32r))
                nc.sync.dma_start(wf[h][:], wabcd_d[128 * h : 128 * (h + 1), 768:1024])
            ppA = P.tile([4, N], f32, name="ppA")
            nc.sync.dma_start(ppA[:], pp_d[0:4, :])
            ppB = P.tile([4, N], f32, name="ppB")
            nc.sync.dma_start(ppB[:], pp_d[4:8, :])
            sw = P.tile([4, D], f32, name="sw")
            nc.sync.dma_start(sw[:], sw_d[0:4, :])
            bs = P.tile([128, 6], f32, name="bs")
            nc.sync.dma_start(bs[:], bs_d[:])

            qT = [P.tile([128, N], f32, name=f"qT{h}") for h in range(2)]
            kT = [P.tile([128, N], f32, name=f"kT{h}") for h in range(2)]
            vT = [P.tile([128, N], f32, name=f"vT{h}") for h in range(2)]
            uT = [P.tile([128, N], f32, name=f"uT{h}") for h in range(2)]
            dest = [qT, kT, vT]

            # Phase A: q/k/v projections (f32r) and u = spw1 @ pos (fp32 exact)
            for o in range(3):
                for h in range(2):
                    col = o * 256 + h * 128
                    for nf in range(4):
                        pA = PS.tile([128, 512], f32, name="pA", tag="psA", bufs=2)
                        nc.tensor.matmul(
                            pA[:], wq[0][:, col : col + 128],
                            xs[0][:, nf * 512 : (nf + 1) * 512],
                            start=True, stop=False,
                        )
                        nc.tensor.matmul(
                            pA[:], wq[1][:, col : col + 128],
                            xs[1][:, nf * 512 : (nf + 1) * 512],
                            start=False, stop=True,
                        )
                        nc.scalar.copy(dest[o][h][:, nf * 512 : (nf + 1) * 512], pA[:])
            for h in range(2):
                for nf in range(4):
                    pU = PS.tile([128, 512], f32, name="pU", tag="psA", bufs=2)
                    nc.tensor.matmul(
                        pU[:], sw[:, h * 128 : (h + 1) * 128],
                        ppA[:, nf * 512 : (nf + 1) * 512],
                        start=True, stop=True,
                    )
                    nc.scalar.copy(uT[h][:, nf * 512 : (nf + 1) * 512], pU[:])

            idxts = [P.tile([128, 128], u16, name=f"idxt{rt}") for rt in range(16)]

            for rt in range(16):
                # Phase B: neighbor scores for 128 query tokens (exact fp32)
                # score[n, m] = 2*pos[n].pos[m] - sq[m]  (same ranking as -dist)
                Gs = R.tile([128, 2048], f32, name="Gs")
                for bb in range(4):
                    Gp = PS.tile([128, 512], f32, name="Gp", tag="psA", bufs=2)
                    nc.tensor.matmul(
                        Gp[:],
                        ppB[:, rt * 128 : (rt + 1) * 128],
                        ppA[:, bb * 512 : (bb + 1) * 512],
                        start=True, stop=True,
                    )
                    nc.scalar.copy(Gs[:, bb * 512 : (bb + 1) * 512], Gp[:])
                mxt = R.tile([128, 16], f32, name="mxt")
                mip = R.tile([128, 128], u16, name="mip")
                nc.vector.memset(mip[:], 0)
                nc.vector.max(mxt[:, 0:8], Gs[:])
                nc.vector.max_index(mip[:, 0:8], mxt[:, 0:8], Gs[:])
                Gm = R.tile([128, 2048], f32, name="Gm")
                nc.vector.match_replace(Gm[:], mxt[:, 0:8], Gs[:], -3e38)
                nc.vector.max(mxt[:, 8:16], Gm[:])
                nc.vector.max_index(mip[:, 8:16], mxt[:, 8:16], Gm[:])
                it = idxts[rt]
                nc.sync.dma_start_transpose(it[:], mip[:])
                for g in range(1, 8):
                    nc.sync.dma_start(it[16 * g : 16 * (g + 1), :], it[0:16, :])

                # Phase D: pair loop over this row-tile's chunks
                for sub in range(2):
                    c = 2 * rt + sub
                    t0 = c * CH
                    idx_ap = it[:, sub * CH : (sub + 1) * CH].bitcast(i16)
                    kg, vg, ug = [], [], []
                    for h in range(2):
                        kgh = GP.tile([128, CH * K], f32, name=f"kg{h}")
                        nc.gpsimd.ap_gather(
                            kgh[:], kT[h][:], idx_ap,
                            channels=128, num_elems=N, d=1, num_idxs=CH * K,
                        )
                        kg.append(kgh)
                        vgh = GP.tile([128, CH * K], f32, name=f"vg{h}")
                        nc.gpsimd.ap_gather(
                            vgh[:], vT[h][:], idx_ap,
                            channels=128, num_elems=N, d=1, num_idxs=CH * K,
                        )
                        vg.append(vgh)
                        ugh = GP.tile([128, CH * K], f32, name=f"ug{h}")
                        nc.gpsimd.ap_gather(
                            ugh[:], uT[h][:], idx_ap,
                            channels=128, num_elems=N, d=1, num_idxs=CH * K,
                        )
                        ug.append(ugh)
                    Z = [CP.tile([128, CH], f32, name=f"Z{h}") for h in range(2)]
                    agg = [CP.tile([128, CH], f32, name=f"agg{h}") for h in range(2)]
                    for fc in range(CH * K // FC):
                        pr = slice(fc * FC, (fc + 1) * FC)
                        tl = slice(fc * TPF, (fc + 1) * TPF)
                        tg = slice(t0 + fc * TPF, t0 + (fc + 1) * TPF)
                        qmk, pe1 = [], []
                        for h in range(2):
                            qm = FP.tile([128, FC], f32, name=f"qm{h}")
                            nc.vector.tensor_sub(
                                qm[:].rearrange("p (a b) -> p a b", b=K),
                                qT[h][:, tg].unsqueeze(2).broadcast_to([128, TPF, K]),
                                kg[h][:, pr].rearrange("p (a b) -> p a b", b=K),
                            )
                            qmk.append(qm)
                            du = FP.tile([128, FC], f32, name=f"du{h}")
                            nc.vector.tensor_sub(
                                du[:].rearrange("p (a b) -> p a b", b=K),
                                uT[h][:, tg].unsqueeze(2).broadcast_to([128, TPF, K]),
                                ug[h][:, pr].rearrange("p (a b) -> p a b", b=K),
                            )
                            p1 = FP.tile([128, FC], f32r, name=f"pe1{h}")
                            nc.scalar.activation(
                                p1[:], du[:], AF.Relu, bias=bs[:, h : h + 1], scale=1.0
                            )
                            pe1.append(p1)
                        pe_ps = []
                        for h in range(2):
                            pps = PS.tile([128, FC], f32, name=f"pps{h}", tag=f"pps{h}")
                            nc.tensor.matmul(
                                pps[:], wA[0][:, h * 128 : (h + 1) * 128], pe1[0][:],
                                start=True, stop=False,
                            )
                            nc.tensor.matmul(
                                pps[:], wA[1][:, h * 128 : (h + 1) * 128], pe1[1][:],
                                start=False, stop=True,
                            )
                            pe_ps.append(pps)
                        qkpe = []
                        for h in range(2):
                            qp = FP.tile([128, FC], f32r, name=f"qkpe{h}")
                            nc.vector.tensor_add(qp[:], qmk[h][:], pe_ps[h][:])
                            qkpe.append(qp)
                        a1r = []
                        for h in range(2):
                            a1p = PS.tile([128, FC], f32, name=f"a1p{h}", tag=f"a1p{h}")
                            nc.tensor.matmul(
                                a1p[:], wA[0][:, 256 + h * 128 : 256 + (h + 1) * 128],
                                qkpe[0][:], start=True, stop=False,
                            )
                            nc.tensor.matmul(
                                a1p[:], wA[1][:, 256 + h * 128 : 256 + (h + 1) * 128],
                                qkpe[1][:], start=False, stop=True,
                            )
                            ar = FP.tile([128, FC], f32r, name=f"a1r{h}")
                            nc.scalar.activation(
                                ar[:], a1p[:], AF.Relu, bias=bs[:, 2 + h : 3 + h], scale=1.0
                            )
                            a1r.append(ar)
                        for h in range(2):
                            a2p = PS.tile([128, FC], f32, name=f"a2p{h}", tag=f"a2p{h}")
                            nc.tensor.matmul(
                                a2p[:], wA[0][:, 512 + h * 128 : 512 + (h + 1) * 128],
                                a1r[0][:], start=True, stop=False,
                            )
                            nc.tensor.matmul(
                                a2p[:], wA[1][:, 512 + h * 128 : 512 + (h + 1) * 128],
                                a1r[1][:], start=False, stop=True,
                            )
                            eh = FP.tile([128, FC], f32, name=f"eh{h}")
                            nc.scalar.activation(eh[:], a2p[:], AF.Exp, bias=0.0, scale=1.0)
                            nc.vector.tensor_reduce(
                                Z[h][:, tl],
                                eh[:].rearrange("p (a b) -> p a b", b=K),
                                AX.X, OP.add,
                            )
                            nc.vector.tensor_add(vg[h][:, pr], vg[h][:, pr], pe_ps[h][:])
                            nc.vector.tensor_mul(vg[h][:, pr], eh[:], vg[h][:, pr])
                            nc.vector.tensor_reduce(
                                agg[h][:, tl],
                                vg[h][:, pr].rearrange("p (a b) -> p a b", b=K),
                                AX.X, OP.add,
                            )
                    aggn = []
                    for h in range(2):
                        rz = CP.tile([128, CH], f32, name=f"rz{h}")
                        nc.vector.reciprocal(rz[:], Z[h][:])
                        an = CP.tile([128, CH], f32, name=f"an{h}")
                        nc.vector.tensor_mul(an[:], agg[h][:], rz[:])
                        aggn.append(an)
                    for h in range(2):
                        op_ = PS.tile([128, CH], f32, name=f"op{h}", tag=f"pps{h}",
                                      padded_shape=[128, FC])
                        nc.tensor.matmul(
                            op_[:], wf[0][:, h * 128 : (h + 1) * 128], aggn[0][:],
                            start=True, stop=False,
                        )
                        nc.tensor.matmul(
                            op_[:], wf[1][:, h * 128 : (h + 1) * 128], aggn[1][:],
                            start=False, stop=True,
                        )
                        ob = CP.tile([128, CH], f32, name=f"ob{h}")
                        nc.scalar.activation(
                            ob[:], op_[:], AF.Identity, bias=bs[:, 4 + h : 5 + h], scale=1.0
                        )
                        nc.vector.tensor_add(
                            ob[:], ob[:], xs[h][:, t0 : t0 + CH].bitcast(f32)
                        )
                        nc.sync.dma_start(outT_d[h * 128 : (h + 1) * 128, t0 : t0 + CH], ob[:])

    nc.compile()
    return nc


def _get_nc():
    global _NC
    if _NC is None:
        _NC = _build()
    return _NC


def _make_in_maps(inputs):
    f = lambda k: np.ascontiguousarray(np.asarray(inputs[k], dtype=np.float32))
    x, pos = f("x"), f("pos")
    Wq, Wk, Wv, Wf, bf = f("Wq"), f("Wk"), f("Wv"), f("Wf"), f("bf")
    pm_w1, pm_g1, pm_b1 = f("pm_w1"), f("pm_g1"), f("pm_b1")
    pm_m1, pm_v1, pm_w2 = f("pm_m1"), f("pm_v1"), f("pm_w2")
    am_w1, am_g1, am_b1 = f("am_w1"), f("am_g1"), f("am_b1")
    am_m1, am_v1, am_w2 = f("am_m1"), f("am_v1"), f("am_w2")

    scale1 = pm_g1 / np.sqrt(pm_v1 + EPS)
    spw1 = pm_w1 * scale1[:, None]
    bias1 = pm_b1 - pm_m1 * scale1
    scaleA = am_g1 / np.sqrt(am_v1 + EPS)
    sam_w1 = am_w1 * scaleA[:, None]
    biasA = am_b1 - am_m1 * scaleA

    wqkv = np.ascontiguousarray(
        np.concatenate([Wq.T, Wk.T, Wv.T], axis=1), dtype=np.float32
    )
    wabcd = np.ascontiguousarray(
        np.concatenate([pm_w2.T, sam_w1.T, (am_w2 / 16.0).T, Wf.T], axis=1),
        dtype=np.float32,
    )
    spw1T = np.zeros((16, D), np.float32)
    spw1T[0:3, :] = spw1.T
    biases = np.zeros((128, 6), np.float32)
    biases[:, 0], biases[:, 1] = bias1[:128], bias1[128:]
    biases[:, 2], biases[:, 3] = biasA[:128], biasA[128:]
    biases[:, 4], biases[:, 5] = bf[:128], bf[128:]

    in_maps = []
    for b in range(B):
        xT = np.ascontiguousarray(x[b].T)
        p = pos[b]
        sq = (p * p).sum(-1)
        pospack = np.zeros((16, N), np.float32)
        pospack[0:3, :] = p.T
        pospack[3, :] = -sq
        pospack[4:7, :] = 2.0 * p.T
        pospack[7, :] = 1.0
        in_maps.append(
            {
                "xT": xT,
                "pospack": pospack,
                "wqkv": wqkv,
                "wabcd": wabcd,
                "spw1T": spw1T,
                "biases": biases,
            }
        )
    return in_maps


def kernel(**inputs):
    nc = _get_nc()
    in_maps = _make_in_maps(inputs)
    res = bass_utils.run_bass_kernel_spmd(nc, in_maps, list(range(B)), trace=False)
    out = np.stack([np.asarray(res.results[b]["outT"]).T for b in range(B)])
    return np.ascontiguousarray(out.astype(np.float32))

